# revision 18
# baseline (speedup 1.0000x reference)
"""FlowNet correlation kernel for Trainium2 (Bass/Tile), 8-core data-parallel.

out[b, j*21+i, y, x] = (1/C) * sum_c x1[b,c,y,x] * pad20(x2)[b,c, y+2j, x+2i]

Strategy (per core = one batch element):
  - Parity-split y and x (displacement stride 2) -> 4 independent parity
    sub-problems; block pairs (y,x) into 128-partition stationary tiles
    (RY=8 parity-rows x RX=16 parity-cols).
  - PE computes the banded Gram rectangle per block with a float32r matmul:
    psum[pair, (a,b)] = <x1[:,pair], x2[:, halo(a,b)]>, halo 28x36.
  - Escape PSUM->SBUF with the 1/C scale (split across DVE and ACT).
  - Sheared-AP DMA gathers each pair's 21x21 displacement window:
    E[pair, j*21+i] = rect[pair, (dy'+j, dx'+i)]  (per-partition affine base).
  - PE transpose (is_transpose matmul vs identity) flips to [ji, pair].
  - DVE merge-copy interleaves x-parities into [ji, y-rows, x] out tiles.
  - DMA out with 640B contiguous rows.
"""

import numpy as np

import concourse.bacc as bacc
import concourse.bass as bass
import concourse.mybir as mybir
import concourse.tile as tile
from concourse.bass_utils import run_bass_kernel_spmd
from concourse.masks import make_identity

F32 = mybir.dt.float32
F32R = mybir.dt.float32r
BF16 = mybir.dt.bfloat16

C = 256
H = 96
W = 160
NB = 8
J = 21          # taps per axis
PAD = 20
RY = 8          # parity rows per block
RX = 16         # parity cols per block
JI = J * J      # 441
JIPAD = 448
CHW = 112       # fold chunk width (JIPAD // 4)


def build_nc(h=H, w=W, n_cores=NB):
    hp, wp = h // 2, w // 2
    gys, gxs = hp // RY, wp // RX
    ah, bw = RY + PAD, RX + PAD       # halo extents (28, 36)
    rect = ah * bw                    # 1008
    hw = h * w

    nc = bacc.Bacc("TRN2", target_bir_lowering=False, debug=False,
                   num_devices=n_cores)
    x1d = nc.dram_tensor("input1", [C, h, w], F32, kind="ExternalInput")
    x2d = nc.dram_tensor("input2", [C, h, w], F32, kind="ExternalInput")
    outd = nc.dram_tensor("out", [JI, h, w], F32, kind="ExternalOutput")

    with tile.TileContext(nc) as tc:
        with (
            tc.tile_pool(name="x2pool", bufs=1) as x2pool,
            tc.tile_pool(name="x1pool", bufs=2) as x1pool,
            tc.tile_pool(name="identpool", bufs=1) as identpool,
            tc.tile_pool(name="rectpool", bufs=2) as rectpool,
            tc.tile_pool(name="epool", bufs=2) as epool,
            tc.tile_pool(name="outpool", bufs=4) as outpool,
            tc.tile_pool(name="dramscr", bufs=4, space="DRAM") as dramscr,
            tc.tile_pool(name="rectps", bufs=2, space="PSUM") as rectps,
            tc.tile_pool(name="foldps", bufs=4, space="PSUM") as foldps,
        ):
            ident = identpool.tile([128, 128], BF16)
            make_identity(nc, ident[:])

            x2sb = x2pool.tile([128, 2, h, w], F32R)
            for k in range(2):
                nc.sync.dma_start(out=x2sb[:, k],
                                  in_=x2d[k * 128:(k + 1) * 128]
                                  .bitcast(F32R))

            for py in range(2):
                for gy in range(gys):
                    y0 = py + 2 * RY * gy            # first real y row
                    # x1 rows for this group: y = y0 + 2*dy'
                    # x1 rows for this group (contiguous DMA), then a
                    # gpsimd rearrange into block-major stationary tiles:
                    # walrus needs matmul weights APs to have ONE free dim.
                    x1t = x1pool.tile([128, 2, RY, w], F32, tag="x1t",
                                      bufs=1)
                    for k in range(2):
                        src = bass.AP(
                            tensor=x1d, offset=k * 128 * hw + y0 * w,
                            ap=[[hw, 128], [2 * w, RY], [1, w]])
                        nc.sync.dma_start(out=x1t[:, k], in_=src)
                    x1s = x1pool.tile([128, 2, 2, gxs, RY * RX], F32R,
                                      tag="x1s")
                    for k in range(2):
                        for px in range(2):
                            for gx in range(gxs):
                                x0 = px + 2 * RX * gx
                                nc.gpsimd.tensor_copy(
                                    out=x1s[:, k, px, gx].rearrange(
                                        "p (a b) -> p a b", a=RY),
                                    in_=x1t[:, k, :, x0:x0 + 2 * RX - 1:2]
                                    .bitcast(F32R))

                    ots = [outpool.tile([CHW, RY * w], F32, tag="ot",
                                        name=f"ot{py}_{gy}_{ci}")
                           for ci in range(4)]

                    for px in range(2):
                        for gx in range(gxs):
                            x0 = px + 2 * RX * gx
                            # valid halo ranges (rows r = y0 + 2a - 20,
                            # cols u = x0 + 2b - 20)
                            alo = max(0, -(-(PAD - y0) // 2))
                            ahi = min(ah, (h - 1 - y0 + PAD) // 2 + 1)
                            blo = max(0, -(-(PAD - x0) // 2))
                            bhi = min(bw, (w - 1 - x0 + PAD) // 2 + 1)
                            nb_ = bhi - blo

                            # psum rect in two bank-aligned halves:
                            # half hh holds a in [14hh, 14hh+14) at
                            # elem offsets [512hh, 512hh + 14*bw)
                            rp = rectps.tile([128, 2, 512], F32, tag="rp")
                            rs = rectpool.tile([128, ah, bw], BF16,
                                                               tag="rs")

                            # zero-fill clipped halo strips in SBUF rect
                            if alo > 0:
                                nc.gpsimd.memset(rs[:, :alo, :], 0.0)
                            if ahi < ah:
                                nc.gpsimd.memset(rs[:, ahi:, :], 0.0)
                            if blo > 0:
                                nc.gpsimd.memset(rs[:, alo:ahi, :blo], 0.0)
                            if bhi < bw:
                                nc.gpsimd.memset(rs[:, alo:ahi, bhi:], 0.0)

                            # banded Gram matmuls, K=256 in two 128-chunks,
                            # one matmul per psum-bank half per K-chunk
                            rpap = rp[:]
                            hranges = []
                            for hh in range(2):
                                a0 = max(alo, 14 * hh)
                                a1 = min(ahi, 14 * (hh + 1))
                                if a0 >= a1:
                                    continue
                                hranges.append((hh, a0, a1))
                                na = a1 - a0
                                pout = bass.AP(
                                    tensor=rpap.tensor,
                                    offset=rpap.offset + 512 * hh,
                                    ap=[[1024, 128], [1, na * nb_]])
                                for k in range(2):
                                    lhsT = x1s[:, k, px, gx]
                                    r0 = y0 + 2 * a0 - PAD
                                    u0 = x0 + 2 * blo - PAD
                                    rhs = x2sb[:, k,
                                               r0:r0 + 2 * na - 1:2,
                                               u0:u0 + 2 * nb_ - 1:2]
                                    nc.tensor.matmul(
                                        pout, lhsT, rhs,
                                        start=(k == 0), stop=(k == 1))

                            # escape PSUM -> SBUF with 1/C scale
                            # (half 0 on DVE, half 1 on ACT)
                            for hh, a0, a1 in hranges:
                                na = a1 - a0
                                psrc = bass.AP(
                                    tensor=rpap.tensor,
                                    offset=rpap.offset + 512 * hh,
                                    ap=[[1024, 128], [nb_, na], [1, nb_]])
                                if hh == 0:
                                    nc.vector.tensor_scalar_mul(
                                        rs[:, a0:a1, blo:bhi], psrc, 1.0 / C)
                                else:
                                    nc.scalar.mul(
                                        rs[:, a0:a1, blo:bhi], psrc, 1.0 / C)

                            # gather via DRAM bounce (flat DRAM strides are
                            # unrestricted; SBUF partition steps must be
                            # whole rows). Leg 1: per dy'-group, the rows
                            # [dy', dy'+21) x 36 are one contiguous slab.
                            scr = dramscr.tile([128, J * bw], BF16, tag="scr")
                            scrap = scr[:]
                            rsap = rs[:]
                            for dy in range(RY):
                                ssrc = bass.AP(
                                    tensor=rsap.tensor,
                                    offset=rsap.offset
                                    + dy * (RX * rect + bw),
                                    ap=[[rect, RX], [1, J * bw]])
                                sdst = bass.AP(
                                    tensor=scrap.tensor,
                                    offset=scrap.offset + dy * RX * J * bw,
                                    ap=[[J * bw, RX], [1, J * bw]])
                                nc.sync.dma_start(out=sdst, in_=ssrc)
                            # Leg 2: read back with the dx' shear done on the
                            # DRAM side -> compact E[pair, j*21+i].
                            et = epool.tile([128, JIPAD], BF16, tag="et")
                            nc.vector.memset(et[:, JI:], 0.0)
                            eap = et[:]
                            for dy in range(RY):
                                gsrc = bass.AP(
                                    tensor=scrap.tensor,
                                    offset=scrap.offset + dy * RX * J * bw,
                                    ap=[[J * bw + 1, RX], [bw, J], [1, J]])
                                gdst = bass.AP(
                                    tensor=eap.tensor,
                                    offset=eap.offset + dy * RX * JIPAD,
                                    ap=[[JIPAD, RX], [J, J], [1, J]])
                                nc.sync.dma_start(out=gdst, in_=gsrc)

                            # PE transpose chunks + merge into out tiles
                            for ci in range(4):
                                nj = min(CHW, JI - ci * CHW)
                                fp = foldps.tile([CHW, 128], BF16, tag="fp")
                                nc.tensor.transpose(
                                    fp[:],
                                    et[:, ci * CHW:(ci + 1) * CHW],
                                    ident[:])
                                dst = bass.AP(
                                    tensor=ots[ci][:].tensor,
                                    offset=ots[ci][:].offset + x0,
                                    ap=[[RY * w, nj], [w, RY], [2, RX]])
                                nc.vector.tensor_copy(
                                    out=dst, in_=fp[:nj].rearrange(
                                        "p (a b) -> p a b", a=RY))

                    # DMA out: per fold chunk
                    for ci in range(4):
                        nj = min(CHW, JI - ci * CHW)
                        dst = bass.AP(
                            tensor=outd, offset=ci * CHW * hw + y0 * w,
                            ap=[[hw, nj], [2 * w, RY], [1, w]])
                        nc.sync.dma_start(out=dst, in_=ots[ci][:nj])

    nc.compile()
    return nc


_NC_CACHE = {}


def _get_nc(h, w, n_cores):
    key = (h, w, n_cores)
    if key not in _NC_CACHE:
        _NC_CACHE[key] = build_nc(h, w, n_cores)
    return _NC_CACHE[key]


def kernel(input1, input2):
    input1 = np.asarray(input1, dtype=np.float32)
    input2 = np.asarray(input2, dtype=np.float32)
    b, c, h, w = input1.shape
    assert c == C
    nc = _get_nc(h, w, b)
    in_maps = [
        {"input1": np.ascontiguousarray(input1[i]),
         "input2": np.ascontiguousarray(input2[i])}
        for i in range(b)
    ]
    res = run_bass_kernel_spmd(nc, in_maps, core_ids=list(range(b)))
    return np.stack([res.results[i]["out"] for i in range(b)])


# revision 22
# speedup vs baseline: 1.6684x; 1.6684x over previous
"""FlowNet correlation kernel for Trainium2 (Bass/Tile), 8-core data-parallel.

out[b, j*21+i, y, x] = (1/C) * sum_c x1[b,c,y,x] * pad20(x2)[b,c, y+2j, x+2i]

Strategy (per core = one batch element):
  - Parity-split y and x (displacement stride 2); block pairs (y,x) into
    128-partition stationary tiles (RY=8 parity-rows x RX=16 parity-cols),
    pair index p = dy*RX + dx (dy-major).
  - PE computes the banded Gram rectangle per block in bf16:
    psum[pair, (a,b)] = <x1[:,pair], x2[:, halo(a,b)]>, halo 28x36.
  - Escape PSUM->SBUF bf16 with the 1/C scale (split DVE/ACT).
  - Gather of each pair's 21x21 window bounces through DRAM (flat DRAM
    strides are unrestricted; SBUF partition steps must be whole rows):
    8 slab writes/block with a gamma=287 dx-shear makes the per-pair
    read base exactly (S+36)*p, so the compacting read back is ONE
    3-dim DMA per block.
  - PE transpose (vs bf16 identity) flips E to [ji, pair] chunks.
  - DVE merge-copy interleaves x-parities into fp32 [ji, y, x] out tiles.
  - DMA out with 640B contiguous rows.
  - DMA instructions alternate between the two HWDGE rings (sync/scalar);
    input casts fp32->bf16 ride SWDGE (gpsimd) DMAs.
"""

import numpy as np

import concourse.bacc as bacc
import concourse.bass as bass
import concourse.mybir as mybir
import concourse.tile as tile
from concourse.bass_utils import run_bass_kernel_spmd
from concourse.masks import make_identity

F32 = mybir.dt.float32
BF16 = mybir.dt.bfloat16

C = 256
H = 96
W = 160
NB = 8
J = 21          # taps per axis
PAD = 20
RY = 8          # parity rows per block
RX = 16         # parity cols per block
JI = J * J      # 441
JIPAD = 448
CHW = 112       # fold chunk width (JIPAD // 4)
SLAB = J * (RX + PAD)   # 756 contiguous elems per pair slab
# Slab for pair p=(dy*RX+dx) holds rect rows [dy, dy+21); within it,
# E[p, (j,i)] sits at j*36 + dx + i. Storing the slab at base S*p - dx
# makes the read-back offset exactly S*p + 36*j + i -> one 3-dim DMA.
S = 768         # scratch stride per pair (>= SLAB + 1)


def build_nc(h=H, w=W, n_cores=NB):
    hp, wp = h // 2, w // 2
    gys, gxs = hp // RY, wp // RX
    ah, bw = RY + PAD, RX + PAD       # halo extents (28, 36)
    rect = ah * bw                    # 1008
    hw = h * w

    nc = bacc.Bacc("TRN2", target_bir_lowering=False, debug=False,
                   num_devices=n_cores)
    x1d = nc.dram_tensor("input1", [C, h, w], F32, kind="ExternalInput")
    x2d = nc.dram_tensor("input2", [C, h, w], F32, kind="ExternalInput")
    outd = nc.dram_tensor("out", [JI, h, w], F32, kind="ExternalOutput")

    hwdge = [nc.sync, nc.scalar]      # the two HWDGE rings

    with tile.TileContext(nc) as tc:
        with (
            tc.tile_pool(name="x2pool", bufs=1) as x2pool,
            tc.tile_pool(name="x1pool", bufs=2) as x1pool,
            tc.tile_pool(name="identpool", bufs=1) as identpool,
            tc.tile_pool(name="rectpool", bufs=3) as rectpool,
            tc.tile_pool(name="epool", bufs=3) as epool,
            tc.tile_pool(name="outpool", bufs=6) as outpool,
            tc.tile_pool(name="dramscr", bufs=4, space="DRAM") as dramscr,
            tc.tile_pool(name="rectps", bufs=2, space="PSUM") as rectps,
            tc.tile_pool(name="foldps", bufs=4, space="PSUM") as foldps,
        ):
            ident = identpool.tile([128, 128], BF16)
            make_identity(nc, ident[:])

            x2sb = x2pool.tile([128, 2, h, w], BF16)
            for k in range(2):
                nc.gpsimd.dma_start(out=x2sb[:, k],
                                    in_=x2d[k * 128:(k + 1) * 128])

            blk = 0
            for py in range(2):
                for gy in range(gys):
                    y0 = py + 2 * RY * gy            # first real y row
                    # x1 rows for this group (cast to bf16 on load), then a
                    # gpsimd rearrange into block-major stationary tiles
                    # (walrus: matmul weights APs must have ONE free dim).
                    x1t = x1pool.tile([128, 2, RY, w], BF16, tag="x1t",
                                      bufs=2)
                    for k in range(2):
                        src = bass.AP(
                            tensor=x1d, offset=k * 128 * hw + y0 * w,
                            ap=[[hw, 128], [2 * w, RY], [1, w]])
                        nc.gpsimd.dma_start(out=x1t[:, k], in_=src)
                    x1s = x1pool.tile([128, 2, 2, gxs, RY * RX], BF16,
                                      tag="x1s")
                    for k in range(2):
                        for px in range(2):
                            for gx in range(gxs):
                                x0 = px + 2 * RX * gx
                                src = x1t[:, k, :, x0:x0 + 2 * RX - 1:2]
                                nc.gpsimd.tensor_copy(
                                    out=x1s[:, k, px, gx].rearrange(
                                        "p (a b) -> p a b", a=RY),
                                    in_=src)

                    ots = [outpool.tile([CHW, RY * w], F32, tag="ot",
                                        name=f"ot{py}_{gy}_{ci}")
                           for ci in range(4)]

                    for px in range(2):
                        for gx in range(gxs):
                            eng = hwdge[blk % 2]
                            blk += 1
                            x0 = px + 2 * RX * gx
                            # valid halo ranges (rows r = y0 + 2a - 20,
                            # cols u = x0 + 2b - 20)
                            alo = max(0, -(-(PAD - y0) // 2))
                            ahi = min(ah, (h - 1 - y0 + PAD) // 2 + 1)
                            blo = max(0, -(-(PAD - x0) // 2))
                            bhi = min(bw, (w - 1 - x0 + PAD) // 2 + 1)
                            nb_ = bhi - blo

                            # psum rect in two bank-aligned halves: half hh
                            # holds a in [14hh, 14hh+14) at [512hh, ...)
                            rp = rectps.tile([128, 2, 512], F32, tag="rp")
                            rs = rectpool.tile([128, ah, bw], BF16,
                                               tag="rs")

                            # zero-fill clipped halo strips in SBUF rect
                            if alo > 0:
                                nc.gpsimd.memset(rs[:, :alo, :], 0.0)
                            if ahi < ah:
                                nc.gpsimd.memset(rs[:, ahi:, :], 0.0)
                            if blo > 0:
                                nc.gpsimd.memset(rs[:, alo:ahi, :blo], 0.0)
                            if bhi < bw:
                                nc.gpsimd.memset(rs[:, alo:ahi, bhi:], 0.0)

                            # banded Gram matmuls, K=256 in two 128-chunks,
                            # one matmul per psum-bank half per K-chunk
                            rpap = rp[:]
                            hranges = []
                            for hh in range(2):
                                a0 = max(alo, 14 * hh)
                                a1 = min(ahi, 14 * (hh + 1))
                                if a0 >= a1:
                                    continue
                                hranges.append((hh, a0, a1))
                                na = a1 - a0
                                pout = bass.AP(
                                    tensor=rpap.tensor,
                                    offset=rpap.offset + 512 * hh,
                                    ap=[[1024, 128], [1, na * nb_]])
                                for k in range(2):
                                    lhsT = x1s[:, k, px, gx]
                                    r0 = y0 + 2 * a0 - PAD
                                    u0 = x0 + 2 * blo - PAD
                                    rhs = x2sb[:, k,
                                               r0:r0 + 2 * na - 1:2,
                                               u0:u0 + 2 * nb_ - 1:2]
                                    nc.tensor.matmul(
                                        pout, lhsT, rhs,
                                        start=(k == 0), stop=(k == 1))

                            # escape PSUM -> SBUF bf16 with 1/C scale
                            # (half 0 on DVE, half 1 on ACT)
                            for hh, a0, a1 in hranges:
                                na = a1 - a0
                                psrc = bass.AP(
                                    tensor=rpap.tensor,
                                    offset=rpap.offset + 512 * hh,
                                    ap=[[1024, 128], [nb_, na], [1, nb_]])
                                if hh == 0:
                                    nc.vector.tensor_scalar_mul(
                                        rs[:, a0:a1, blo:bhi], psrc, 1.0 / C)
                                else:
                                    nc.scalar.mul(
                                        rs[:, a0:a1, blo:bhi], psrc, 1.0 / C)

                            # gather via DRAM bounce. Leg 1: per dy-group g,
                            # partitions [16g, 16g+16) share slab rows
                            # [g, g+21) (contiguous 756); stored at base
                            # S*p - dx so leg 2 reads at S*p + 36j + i.
                            scr = dramscr.tile([S * 128], BF16, tag="scr")
                            scrap = scr[:]
                            rsap = rs[:]
                            for g in range(RY):
                                ssrc = bass.AP(
                                    tensor=rsap.tensor,
                                    offset=rsap.offset
                                    + RX * g * rect + g * bw,
                                    ap=[[rect, RX], [1, SLAB]])
                                sdst = bass.AP(
                                    tensor=scrap.tensor,
                                    offset=scrap.offset + RX * S * g,
                                    ap=[[S - 1, RX], [1, SLAB]])
                                eng.dma_start(out=sdst, in_=ssrc)
                            # Leg 2: compact read back, one DMA.
                            et = epool.tile([128, JIPAD], BF16, tag="et")
                            nc.vector.memset(et[:, JI:], 0.0)
                            eap = et[:]
                            gsrc = bass.AP(
                                tensor=scrap.tensor, offset=scrap.offset,
                                ap=[[S, 128], [bw, J], [1, J]])
                            gdst = bass.AP(
                                tensor=eap.tensor, offset=eap.offset,
                                ap=[[JIPAD, 128], [J, J], [1, J]])
                            eng.dma_start(out=gdst, in_=gsrc)

                            # PE transpose chunks + merge into out tiles
                            # (pairs dy-major: free dims (dy RX, dx 1))
                            for ci in range(4):
                                nj = min(CHW, JI - ci * CHW)
                                fp = foldps.tile([CHW, 128], BF16, tag="fp")
                                nc.tensor.transpose(
                                    fp[:],
                                    et[:, ci * CHW:(ci + 1) * CHW],
                                    ident[:])
                                fpap = fp[:]
                                msrc = bass.AP(
                                    tensor=fpap.tensor, offset=fpap.offset,
                                    ap=[[128, nj], [RX, RY], [1, RX]])
                                otap = ots[ci][:]
                                mdst = bass.AP(
                                    tensor=otap.tensor,
                                    offset=otap.offset + x0,
                                    ap=[[RY * w, nj], [w, RY], [2, RX]])
                                nc.vector.tensor_copy(out=mdst, in_=msrc)

                    # DMA out: per fold chunk
                    for ci in range(4):
                        nj = min(CHW, JI - ci * CHW)
                        dst = bass.AP(
                            tensor=outd, offset=ci * CHW * hw + y0 * w,
                            ap=[[hw, nj], [2 * w, RY], [1, w]])
                        hwdge[ci % 2].dma_start(out=dst, in_=ots[ci][:nj])

    nc.compile()
    return nc


_NC_CACHE = {}


def _get_nc(h, w, n_cores):
    key = (h, w, n_cores)
    if key not in _NC_CACHE:
        _NC_CACHE[key] = build_nc(h, w, n_cores)
    return _NC_CACHE[key]


def kernel(input1, input2):
    input1 = np.asarray(input1, dtype=np.float32)
    input2 = np.asarray(input2, dtype=np.float32)
    b, c, h, w = input1.shape
    assert c == C
    nc = _get_nc(h, w, b)
    in_maps = [
        {"input1": np.ascontiguousarray(input1[i]),
         "input2": np.ascontiguousarray(input2[i])}
        for i in range(b)
    ]
    res = run_bass_kernel_spmd(nc, in_maps, core_ids=list(range(b)))
    return np.stack([res.results[i]["out"] for i in range(b)])
